# revision 1
# baseline (speedup 1.0000x reference)
"""NeRF positional-encoding kernel for Trainium2 (8 NeuronCores, data parallel).

Problem: x [64, 1024, 4] fp32 in [0,1) -> pe [64, 1024, 768] fp32 where each
row's 768 values are 12 copies of the 64-wide block
  [sin(2^l pi x_d), cos(2^l pi x_d)]  for l in 0..7, d in 0..3,
laid out as block[l*8 + 2*d + (0=sin,1=cos)].

Sharding: batch axis (64) split 8 ways -> per-core x [8192, 4], y [8192, 768].

Per-core kernel strategy (memory-bound: 24 MiB output write per core):
  - rows r = p*64 + q  (p = SBUF partition, q in [0,64))
  - one DMA-in of the whole shard (1 KiB contiguous per partition)
  - for each frequency l: range-reduce m = 2^(l-1)*x exactly with the fp32
    magic-constant round trick, then one Sin activation each for sin/cos
    using the ACT affine pre-scale (sin(2pi f); cos = Sin(2pi g + pi/2),
    g = m - round(m+0.25)), writing interleaved into a [128, 64, 64] block
  - per 16-row chunk: doubling DVE copies replicate the 64-block to 768
    wide, then one big contiguous DMA-out (48 KiB per partition)
"""

import numpy as np

import concourse.mybir as mybir
from concourse.bacc import Bacc
from concourse.tile import TileContext
from concourse.bass_utils import run_bass_kernel_spmd

F32 = mybir.dt.float32
SIN = mybir.ActivationFunctionType.Sin
ALU = mybir.AluOpType

N_CORES = 8
B, T, D = 64, 1024, 4
H = 768
N_FREQS = 8
ROWS = B * T // N_CORES  # 8192 rows per core
P = 128                  # SBUF partitions
Q = ROWS // P            # 64 rows per partition
CH = 16                  # q-rows per output chunk
REP = H // 64            # 12 replicas of the 64-wide block

TWO_PI = float(2.0 * np.pi)
HALF_PI = float(np.pi / 2)
MAGIC = float(1.5 * 2.0**23)  # fp32 round-to-nearest-int for |m| < 2^22

_CACHE = {}


def _build():
    nc = Bacc(trn_type="TRN2", name="nerf_pe")
    x = nc.dram_tensor("x", [ROWS, D], F32, kind="ExternalInput")
    y = nc.dram_tensor("y", [ROWS, H], F32, kind="ExternalOutput")

    with TileContext(nc) as tc:
        with (
            tc.tile_pool(name="main", bufs=1) as pool,
            tc.tile_pool(name="outp", bufs=2) as outp,
        ):
            hp = pool.tile([P, 1], F32)
            tc.nc.vector.memset(hp[:], HALF_PI)

            xin = pool.tile([P, Q * D], F32)
            tc.nc.sync.dma_start(
                xin[:], x[:].rearrange("(p q) d -> p (q d)", p=P)
            )

            block = pool.tile([P, Q, 64], F32)
            for l in range(N_FREQS):
                half = float(2.0 ** (l - 1))  # m = 2^(l-1) * x, exact in fp32
                m = pool.tile([P, Q * D], F32, tag="m", bufs=2)
                t = pool.tile([P, Q * D], F32, tag="t", bufs=2)
                f = pool.tile([P, Q * D], F32, tag="f", bufs=2)
                u = pool.tile([P, Q * D], F32, tag="u", bufs=2)
                g = pool.tile([P, Q * D], F32, tag="g", bufs=2)
                tc.nc.vector.tensor_scalar(m[:], xin[:], half, None, ALU.mult)
                tc.nc.vector.tensor_scalar(
                    t[:], xin[:], half, MAGIC, ALU.mult, ALU.add
                )
                # f = m - (t - MAGIC) = m - round(m)        in [-0.5, 0.5]
                tc.nc.vector.affine_then_add(f[:], t[:], m[:], -1.0, MAGIC)
                tc.nc.vector.tensor_scalar(
                    u[:], m[:], 0.25, MAGIC, ALU.add, ALU.add
                )
                # g = m - round(m + 0.25)                   in [-0.75, 0.25]
                tc.nc.vector.affine_then_add(g[:], u[:], m[:], -1.0, MAGIC)

                # interleaved views: block[.., l*8 + 2d + k], k=0 sin, 1 cos
                sview = block[:, :, l * 8 : (l + 1) * 8].rearrange(
                    "p q (d two) -> p q d two", two=2
                )
                fv = f.rearrange("p (q d) -> p q d", d=D)
                gv = g.rearrange("p (q d) -> p q d", d=D)
                tc.nc.scalar.activation(
                    sview[:, :, :, 0], fv, SIN, scale=TWO_PI
                )
                tc.nc.scalar.activation(
                    sview[:, :, :, 1], gv, SIN, scale=TWO_PI, bias=hp[:]
                )

            yv = y[:].rearrange("(p q) h -> p q h", p=P)
            for cc in range(Q // CH):
                ot = outp.tile([P, CH, H], F32, tag="out")
                qs = slice(cc * CH, (cc + 1) * CH)
                tc.nc.vector.tensor_copy(ot[:, :, 0:64], block[:, qs, :])
                tc.nc.vector.tensor_copy(ot[:, :, 64:128], ot[:, :, 0:64])
                tc.nc.vector.tensor_copy(ot[:, :, 128:256], ot[:, :, 0:128])
                tc.nc.vector.tensor_copy(ot[:, :, 256:512], ot[:, :, 0:256])
                tc.nc.vector.tensor_copy(ot[:, :, 512:768], ot[:, :, 256:512])
                tc.nc.sync.dma_start(yv[:, qs, :], ot[:])

    nc.finalize()
    return nc


def _get_nc():
    if "nc" not in _CACHE:
        _CACHE["nc"] = _build()
    return _CACHE["nc"]


def kernel(x, _trace=False):
    x = np.ascontiguousarray(np.asarray(x, dtype=np.float32))
    assert x.shape == (B, T, D), x.shape
    nc = _get_nc()
    shards = x.reshape(N_CORES, ROWS, D)
    in_maps = [{"x": np.ascontiguousarray(shards[i])} for i in range(N_CORES)]
    r = run_bass_kernel_spmd(
        nc, in_maps, core_ids=list(range(N_CORES)), trace=_trace
    )
    _CACHE["last_result"] = r
    out = np.stack([r.results[i]["y"] for i in range(N_CORES)])
    return out.reshape(B, T, H)


# revision 5
# speedup vs baseline: 1.1959x; 1.1959x over previous
"""NeRF positional-encoding kernel for Trainium2 (8 NeuronCores, data parallel).

Problem: x [64, 1024, 4] fp32 in [0,1) -> pe [64, 1024, 768] fp32 where each
row's 768 values are 12 copies of the 64-wide block
  [sin(2^l pi x_d), cos(2^l pi x_d)]  for l in 0..7, d in 0..3,
laid out as block[l*8 + 2*d + (0=sin,1=cos)].

Sharding: batch axis (64) split 8 ways -> per-core x [8192, 4], y [8192, 768].

Per-core kernel (memory-bound: 24 MiB output write per core, ~358 GB/s):
  - rows r = p*64 + q (p = SBUF partition, q in [0,64)); output per
    partition is one contiguous 192 KiB HBM range -> ideal DMA descriptors
  - processed in q-chunks [8,16,16,16,8] so the first output DMA starts
    early and streaming hides all compute
  - per chunk: 5 wide DVE ops compute the range reduction for ALL 8
    frequencies at once (0-stride broadcast reads over the freq axis):
      m = 2^(l-1)*x (exact);  f = m - round(m);  g = m - round(m + 0.25)
    using the fp32 magic-constant (1.5*2^23) round trick, then 2 ACT ops
      sin(2 pi m) = Sin(2pi*f),  cos(2 pi m) = Sin(2pi*g + pi/2)
    write interleaved directly into the out tile's first 64 columns
  - 11 independent copies (Vector/Scalar/GpSimd split) replicate to 768
  - one contiguous DMA per chunk streams to HBM
"""

import numpy as np

import concourse.mybir as mybir
from concourse.bacc import Bacc
from concourse.tile import TileContext
from concourse.bass_types import AP
from concourse.bass_utils import run_bass_kernel_spmd

F32 = mybir.dt.float32
SIN = mybir.ActivationFunctionType.Sin
ALU = mybir.AluOpType

N_CORES = 8
B, T, D = 64, 1024, 4
H = 768
L = 8                     # frequencies
P = 128                   # SBUF partitions
ROWS = B * T // N_CORES   # 8192 rows per core
Q = ROWS // P             # 64 rows per partition
CHUNKS = [8, 16, 16, 16, 8]  # q-rows per chunk (sum = Q)

TWO_PI = float(2.0 * np.pi)
HALF_PI = float(np.pi / 2)
MAGIC = float(1.5 * 2.0**23)  # fp32 round-to-nearest-int for |m| < 2^22

_CACHE = {}


def _bcast_x(xsl, w):
    """[128, w] slice of x -> [128, L(stride 0), w] broadcast AP."""
    return AP(xsl.tensor, xsl.offset, [list(xsl.ap[0]), [0, L], [1, w]])


def _bcast_f(fsc, w):
    """[128, L] freq scales -> [128, L, w(stride 0)] broadcast AP."""
    return AP(fsc.tensor, fsc.offset, [list(fsc.ap[0]), list(fsc.ap[1]), [0, w]])


def _build():
    nc = Bacc(trn_type="TRN2", name="nerf_pe")
    x = nc.dram_tensor("x", [ROWS, D], F32, kind="ExternalInput")
    y = nc.dram_tensor("y", [ROWS, H], F32, kind="ExternalOutput")

    # engine split for the 11 replication copies (V=6, A=3, P=2)
    COPY_ENGINES = ["V", "V", "V", "A", "A", "P", "V", "V", "V", "A", "P"]

    def emit_copy(tc, k, out, in_):
        e = COPY_ENGINES[k]
        if e == "V":
            tc.nc.vector.tensor_copy(out, in_)
        elif e == "A":
            tc.nc.scalar.copy(out, in_)
        else:
            tc.nc.gpsimd.tensor_copy(out, in_)

    with TileContext(nc) as tc:
        with (
            tc.tile_pool(name="main", bufs=1) as pool,
            tc.tile_pool(name="outp", bufs=2) as outp,
        ):
            hp = pool.tile([P, 1], F32)
            tc.nc.vector.memset(hp[:], HALF_PI)
            fsc = pool.tile([P, L], F32)
            for l in range(L):
                tc.nc.gpsimd.memset(fsc[:, l : l + 1], float(2.0 ** (l - 1)))

            xin = pool.tile([P, Q * D], F32)
            tc.nc.sync.dma_start(
                xin[:], x[:].rearrange("(p q) d -> p (q d)", p=P)
            )

            yv = y[:].rearrange("(p q) h -> p q h", p=P)

            q0 = 0
            for ci, ch in enumerate(CHUNKS):
                w = ch * D
                xsl = xin[:, q0 * D : (q0 + ch) * D]
                m = pool.tile([P, L, w], F32, tag="m", bufs=2)
                t = pool.tile([P, L, w], F32, tag="t", bufs=2)
                f = pool.tile([P, L, w], F32, tag="f", bufs=2)
                u = pool.tile([P, L, w], F32, tag="u", bufs=2)
                g = pool.tile([P, L, w], F32, tag="g", bufs=2)
                # m = x * 2^(l-1) for all l at once (broadcast reads)
                tc.nc.vector.tensor_tensor(
                    m[:], _bcast_x(xsl, w), _bcast_f(fsc, w), ALU.mult
                )
                tc.nc.vector.tensor_scalar(t[:], m[:], MAGIC, None, ALU.add)
                # f = m - (t - MAGIC) = m - round(m)        in [-0.5, 0.5]
                tc.nc.vector.affine_then_add(f[:], t[:], m[:], -1.0, MAGIC)
                tc.nc.vector.tensor_scalar(
                    u[:], m[:], 0.25, MAGIC, ALU.add, ALU.add
                )
                # g = m - round(m + 0.25)                   in [-0.75, 0.25]
                tc.nc.vector.affine_then_add(g[:], u[:], m[:], -1.0, MAGIC)

                ot = outp.tile(
                    [P, max(CHUNKS), H], F32, tag="out", name=f"ot{ci}"
                )[:, :ch, :]
                # out views: (l, q, d) with strides (8, 768, 2), offset 0/1
                sc_view = ot[:, :, 0:64].rearrange(
                    "p q (l d two) -> p l q d two", l=L, two=2
                )
                fv = f[:].rearrange("p l (q d) -> p l q d", d=D)
                gv = g[:].rearrange("p l (q d) -> p l q d", d=D)
                tc.nc.scalar.activation(
                    sc_view[:, :, :, :, 0], fv, SIN, scale=TWO_PI
                )
                tc.nc.scalar.activation(
                    sc_view[:, :, :, :, 1], gv, SIN, scale=TWO_PI, bias=hp[:]
                )
                for k in range(1, H // 64):
                    emit_copy(
                        tc, k - 1, ot[:, :, 64 * k : 64 * (k + 1)], ot[:, :, 0:64]
                    )
                tc.nc.sync.dma_start(yv[:, q0 : q0 + ch, :], ot[:])
                q0 += ch

    nc.finalize()
    return nc


def _get_nc():
    if "nc" not in _CACHE:
        _CACHE["nc"] = _build()
    return _CACHE["nc"]


def kernel(x, _trace=False):
    x = np.ascontiguousarray(np.asarray(x, dtype=np.float32))
    assert x.shape == (B, T, D), x.shape
    nc = _get_nc()
    shards = x.reshape(N_CORES, ROWS, D)
    in_maps = [{"x": np.ascontiguousarray(shards[i])} for i in range(N_CORES)]
    r = run_bass_kernel_spmd(
        nc, in_maps, core_ids=list(range(N_CORES)), trace=_trace
    )
    _CACHE["last_result"] = r
    out = np.stack([r.results[i]["y"] for i in range(N_CORES)])
    return out.reshape(B, T, H)


# revision 8
# speedup vs baseline: 1.4037x; 1.1738x over previous
"""NeRF positional-encoding kernel for Trainium2 (8 NeuronCores, data parallel).

Problem: x [64, 1024, 4] fp32 in [0,1) -> pe [64, 1024, 768] fp32 where each
row's 768 values are 12 copies of the 64-wide block
  [sin(2^l pi x_d), cos(2^l pi x_d)]  for l in 0..7, d in 0..3,
laid out as block[l*8 + 2*d + (0=sin,1=cos)].

Sharding: batch axis (64) split 8 ways -> per-core x [8192, 4], y [8192, 768].

Per-core kernel (memory-bound: 24 MiB output write per core, ~358 GB/s):
  - rows r = p*64 + q (p = SBUF partition, q in [0,64)); output per
    partition is one contiguous 192 KiB HBM range -> ideal DMA descriptors
  - processed in q-chunks [8,16,16,16,8] so the first output DMA starts
    early and streaming hides all compute
  - per chunk: 5 wide DVE ops compute the range reduction for ALL 8
    frequencies at once (0-stride broadcast reads over the freq axis):
      m = 2^(l-1)*x (exact);  f = m - round(m);  g = m - round(m + 0.25)
    using the fp32 magic-constant (1.5*2^23) round trick, then 2 ACT ops
      sin(2 pi m) = Sin(2pi*f),  cos(2 pi m) = Sin(2pi*g + pi/2)
    write interleaved directly into the out tile's first 64 columns
  - 11 independent copies (Vector/Scalar/GpSimd split) replicate to 768
  - one contiguous DMA per chunk streams to HBM
"""

import numpy as np

import concourse.mybir as mybir
from concourse.bacc import Bacc
from concourse.tile import TileContext
from concourse.bass_types import AP
from concourse.bass_utils import run_bass_kernel_spmd

F32 = mybir.dt.float32
SIN = mybir.ActivationFunctionType.Sin
ALU = mybir.AluOpType

N_CORES = 8
B, T, D = 64, 1024, 4
H = 768
L = 8                     # frequencies
P = 128                   # SBUF partitions
ROWS = B * T // N_CORES   # 8192 rows per core
Q = ROWS // P             # 64 rows per partition
CHUNKS = [4, 8, 12, 20, 20]  # q-rows per chunk (sum = Q)

TWO_PI = float(2.0 * np.pi)
HALF_PI = float(np.pi / 2)
MAGIC = float(1.5 * 2.0**23)  # fp32 round-to-nearest-int for |m| < 2^22

_CACHE = {}


def _bcast_x(xsl, w):
    """[128, w] slice of x -> [128, L(stride 0), w] broadcast AP."""
    return AP(xsl.tensor, xsl.offset, [list(xsl.ap[0]), [0, L], [1, w]])


def _bcast_f(fsc, w):
    """[128, L] freq scales -> [128, L, w(stride 0)] broadcast AP."""
    return AP(fsc.tensor, fsc.offset, [list(fsc.ap[0]), list(fsc.ap[1]), [0, w]])


def _build():
    nc = Bacc(trn_type="TRN2", name="nerf_pe")
    x = nc.dram_tensor("x", [ROWS, D], F32, kind="ExternalInput")
    y = nc.dram_tensor("y", [ROWS, H], F32, kind="ExternalOutput")

    # engine split for the 11 replication copies (V=7, A=4; GpSimd copies
    # measured ~6x slower than DVE — do not use it)
    COPY_ENGINES = ["V", "V", "A", "V", "V", "A", "V", "V", "A", "V", "A"]

    def emit_copy(tc, k, out, in_):
        if COPY_ENGINES[k] == "V":
            tc.nc.vector.tensor_copy(out, in_)
        else:
            tc.nc.scalar.copy(out, in_)

    with TileContext(nc) as tc:
        with (
            tc.tile_pool(name="main", bufs=1) as pool,
            tc.tile_pool(name="outp", bufs=2) as outp,
        ):
            hp = pool.tile([P, 1], F32)
            tc.nc.vector.memset(hp[:], HALF_PI)
            fsc = pool.tile([P, L], F32)
            for l in range(L):
                tc.nc.gpsimd.memset(fsc[:, l : l + 1], float(2.0 ** (l - 1)))

            # chunk-0's input lands via a tiny first DMA so its pipeline
            # starts before the full-shard load completes
            xin = pool.tile([P, Q * D], F32)
            xdram = x[:].rearrange("(p q) d -> p (q d)", p=P)
            w0 = CHUNKS[0] * D
            tc.nc.sync.dma_start(xin[:, 0:w0], xdram[:, 0:w0])
            tc.nc.sync.dma_start(xin[:, w0:], xdram[:, w0:])

            yv = y[:].rearrange("(p q) h -> p q h", p=P)

            q0 = 0
            for ci, ch in enumerate(CHUNKS):
                w = ch * D
                xsl = xin[:, q0 * D : (q0 + ch) * D]
                m = pool.tile([P, L, w], F32, tag="m", bufs=2)
                t = pool.tile([P, L, w], F32, tag="t", bufs=2)
                f = pool.tile([P, L, w], F32, tag="f", bufs=2)
                u = pool.tile([P, L, w], F32, tag="u", bufs=2)
                g = pool.tile([P, L, w], F32, tag="g", bufs=2)
                # m = x * 2^(l-1) for all l at once (broadcast reads)
                tc.nc.vector.tensor_tensor(
                    m[:], _bcast_x(xsl, w), _bcast_f(fsc, w), ALU.mult
                )
                tc.nc.vector.tensor_scalar(t[:], m[:], MAGIC, None, ALU.add)
                # f = m - (t - MAGIC) = m - round(m)        in [-0.5, 0.5]
                tc.nc.vector.affine_then_add(f[:], t[:], m[:], -1.0, MAGIC)
                tc.nc.vector.tensor_scalar(
                    u[:], m[:], 0.25, MAGIC, ALU.add, ALU.add
                )
                # g = m - round(m + 0.25)                   in [-0.75, 0.25]
                tc.nc.vector.affine_then_add(g[:], u[:], m[:], -1.0, MAGIC)

                ot = outp.tile(
                    [P, max(CHUNKS), H], F32, tag="out", name=f"ot{ci}"
                )[:, :ch, :]
                # out views: (l, q, d) with strides (8, 768, 2), offset 0/1
                sc_view = ot[:, :, 0:64].rearrange(
                    "p q (l d two) -> p l q d two", l=L, two=2
                )
                fv = f[:].rearrange("p l (q d) -> p l q d", d=D)
                gv = g[:].rearrange("p l (q d) -> p l q d", d=D)
                tc.nc.scalar.activation(
                    sc_view[:, :, :, :, 0], fv, SIN, scale=TWO_PI
                )
                tc.nc.scalar.activation(
                    sc_view[:, :, :, :, 1], gv, SIN, scale=TWO_PI, bias=hp[:]
                )
                for k in range(1, H // 64):
                    emit_copy(
                        tc, k - 1, ot[:, :, 64 * k : 64 * (k + 1)], ot[:, :, 0:64]
                    )
                tc.nc.sync.dma_start(yv[:, q0 : q0 + ch, :], ot[:])
                q0 += ch

    nc.finalize()
    return nc


def _get_nc():
    if "nc" not in _CACHE:
        _CACHE["nc"] = _build()
    return _CACHE["nc"]


def kernel(x, _trace=False):
    x = np.ascontiguousarray(np.asarray(x, dtype=np.float32))
    assert x.shape == (B, T, D), x.shape
    nc = _get_nc()
    shards = x.reshape(N_CORES, ROWS, D)
    in_maps = [{"x": np.ascontiguousarray(shards[i])} for i in range(N_CORES)]
    r = run_bass_kernel_spmd(
        nc, in_maps, core_ids=list(range(N_CORES)), trace=_trace
    )
    _CACHE["last_result"] = r
    out = np.stack([r.results[i]["y"] for i in range(N_CORES)])
    return out.reshape(B, T, H)
